# revision 21
# baseline (speedup 1.0000x reference)
"""DMMR loss kernel for Trainium2 (8 NeuronCores, data-parallel over patches).

Reference semantics (see problem):
  fp = extract_patches(fixed)   # [3375, 4913]
  mp = extract_patches(moving)  # [3375, 4913]
  keep = (mean(fp == 0, axis=1) <= 0.15)
  out  = tanh(sum((fp @ Wf) * (mp @ Wm), -1))  # [3375]
  value = sum(out * keep) / max(sum(keep), 1)

Sharding: the 3375 patch pairs are split 422-per-core across 8 cores and
padded to 432 columns (16-aligned for the DoubleRow moving AP).  The keep
mask is applied on the host by zeroing the fixed-patch data of dropped
patches (ff=0 -> dot=0 -> tanh=0 contribution, exactly equivalent to
masking); the host also computes the keep count and performs the final
division, so the device computes only sum(tanh(ff . mf)) per shard.

Device layout: K-major fp8 with the contraction dim on SBUF partitions,
DRAM arranged [128, 40 ktiles, 432] so each partition reads contiguous
bytes (3.4KB runs per chunk).  Matmuls use fp8 DoubleRow (K=256 per
instruction): 20 MMs per volume instead of 39.  All data DMAs ride the
Sync HWDGE ring in consumption order (fp chunks then mp chunks); weights
ride the Scalar ring.  Everything stays resident in SBUF, so the rings
drain at full HBM rate and the PE never waits on buffer recycling.
"""

import numpy as np
import ml_dtypes

import concourse.bacc as bacc
import concourse.mybir as mybir
import concourse.tile as tile
from concourse.bass_utils import run_bass_kernel_spmd

PATCH = 17
NPP = 15
N_TOT = NPP**3            # 3375 patches
P3 = PATCH**3             # 4913 elems per patch
F = 64                    # feature dim
N_CORES = 8
NP = 422                  # real patches per core (8*422 = 3376 = 3375 + 1)
NP2 = 432                 # padded to a multiple of 16 (DoubleRow AP step)
KT = 40                   # K tiles of 128 (4913 padded to 5120)
KPAD = KT * 128           # 5120
PAIRS = KT // 2           # 20 DoubleRow pairs (K=256 each)
CT = 8                    # K tiles per DMA chunk (3456B per partition)
NCHUNK = KT // CT         # 5
ZERO_THRESH = 0.15

BF16 = mybir.dt.bfloat16
F32 = mybir.dt.float32
DT = mybir.dt.float8e4
NP_DT = ml_dtypes.float8_e4m3
DR = mybir.MatmulPerfMode.DoubleRow
WARMUP_MM = 4             # throwaway matmuls to pre-warm the PE clock

_COMPILED = None  # cache so repeat kernel() calls reuse the program


def _build_nc():
    nc = bacc.Bacc("TRN2", target_bir_lowering=False, debug=False)

    fpt_d = nc.dram_tensor("fpt", [128, KT, NP2], DT, kind="ExternalInput")
    mpt_d = nc.dram_tensor("mpt", [128, KT, NP2], DT, kind="ExternalInput")
    wf_d = nc.dram_tensor("wf", [128, KT, F], DT, kind="ExternalInput")
    wm_d = nc.dram_tensor("wm", [128, KT, F], DT, kind="ExternalInput")
    out_d = nc.dram_tensor("out", [1, 2], F32, kind="ExternalOutput")

    # All data chunks ride the Sync HWDGE ring in consumption order, so
    # completions arrive in the order the PE consumes them (no FIFO
    # bubbles); weights drain early on the Scalar ring.  K-tile 39 is
    # all padding: its SBUF region is memset instead of transferred.
    # The final pair (19) arrives as two patch-half transfers so half
    # the epilogue chain overlaps the last drain+receipt.
    FP_CHUNKS = [8, 8, 8, 8, 8]
    MP_CHUNKS = [8, 8, 8, 8, 6]     # pairs 0-18; pair 19 is split below
    LAST_TILE = KT - 1  # tile 39: zero-filled, never DMA'd
    WF_SPLIT = 8  # tiles in the first (small) wf piece
    NH = NP2 // 2  # patch-half width (216)

    with tile.TileContext(nc) as tc:
        with (
            tc.tile_pool(name="weights", bufs=1) as wpool,
            tc.tile_pool(name="fdata", bufs=len(FP_CHUNKS)) as fpool,
            tc.tile_pool(name="mdata", bufs=len(MP_CHUNKS)) as mpool,
            tc.tile_pool(name="small", bufs=1) as spool,
            tc.tile_pool(name="psum", bufs=1, space="PSUM") as ppool,
        ):
            # wf arrives in two pieces so the first matmuls only gate on
            # a 64KB transfer; separate tiles give clean dependencies.
            # Everything rides the Sync ring: a single HWDGE ring drains
            # measurably faster than two rings sharing the SDMA engines.
            wfa = wpool.tile([128, WF_SPLIT, F], DT, tag="wfa")
            nc.sync.dma_start(wfa[:], wf_d.ap()[:, :WF_SPLIT, :])

            def wf_pair(t):
                if 2 * t < WF_SPLIT:
                    return wfa[:, 2 * t:2 * t + 2, :]
                return wfb[:, 2 * t - WF_SPLIT:2 * t - WF_SPLIT + 2, :]

            ones_bf = spool.tile([F, 1], BF16, tag="ones_bf")
            nc.vector.memset(ones_bf[:], 1.0)
            junk = spool.tile([128, 2, NP2], DT, tag="junk")
            nc.vector.memset(junk[:], 0.0)

            ps_ff = ppool.tile([F, NP2], F32, tag="ff")
            ps_mf = ppool.tile([F, NP2], F32, tag="mf")
            ps_warm = ppool.tile([F, NP2], F32, tag="warm")

            # pre-warm the PE HAM clock gate during the initial DMA wait
            for w in range(WARMUP_MM):
                nc.tensor.matmul(
                    ps_warm[:],
                    lhsT=junk[:, :, :F],
                    rhs=junk[:],
                    start=(w == 0),
                    stop=(w == WARMUP_MM - 1),
                    perf_mode=DR,
                )

            def stream_volume(chunks, dram, pool, tag, psum, lhsT_of, last_stop,
                              inject=None):
                t = 0
                off = 0
                for ci, ct in enumerate(chunks):
                    ch = pool.tile([128, ct, NP2], DT, tag=tag)
                    if off + ct > LAST_TILE:
                        # last chunk: transfer all but tile 39, memset it
                        nc.sync.dma_start(
                            ch[:, :ct - 1, :],
                            dram.ap()[:, off:off + ct - 1, :],
                        )
                        nc.gpsimd.memset(ch[:, ct - 1:ct, :], 0.0)
                    else:
                        nc.sync.dma_start(
                            ch[:], dram.ap()[:, off:off + ct, :]
                        )
                    if inject and ci in inject:
                        inject[ci]()
                    for s in range(ct // 2):
                        nc.tensor.matmul(
                            psum[:],
                            lhsT=lhsT_of(t),
                            rhs=ch[:, 2 * s:2 * s + 2, :],
                            start=(t == 0),
                            stop=(t == last_stop),
                            perf_mode=DR,
                        )
                        t += 1
                    off += ct

            # wfb lands right behind fp chunk 0 on the ring (needed from
            # pair 4); wm behind chunk 2 (needed only by the mf matmuls)
            wfb = wpool.tile([128, KT - WF_SPLIT, F], DT, tag="wfb")
            wm_sb = wpool.tile([128, KT, F], DT, tag="wm")

            # ---- phase 1: fixed volume (ff DoubleRow matmuls) ----
            stream_volume(
                FP_CHUNKS, fpt_d, fpool, "fp", ps_ff, wf_pair, PAIRS - 1,
                inject={
                    0: lambda: nc.sync.dma_start(
                        wfb[:], wf_d.ap()[:, WF_SPLIT:, :]),
                    2: lambda: nc.sync.dma_start(wm_sb[:], wm_d.ap()),
                },
            )

            # stage ff out of PSUM on the (idle) DVE so the Scalar queue
            # never head-of-line-blocks the scalar ring's descriptor gen
            ff_sb = spool.tile([F, NP2], F32, tag="ff_sb")
            nc.vector.tensor_scalar(
                out=ff_sb[:], in0=ps_ff[:], scalar1=0.0, scalar2=None,
                op0=mybir.AluOpType.add,
            )

            # ---- phase 2: moving volume, pairs 0-18 ----
            stream_volume(MP_CHUNKS, mpt_d, mpool, "mp", ps_mf,
                          lambda t: wm_sb[:, 2 * t:2 * t + 2, :], -1)

            # pair 19 arrives as two patch-half DMAs; each half's epilogue
            # chain (matmul -> prod -> dot -> tanh) runs while the other
            # half (and the output receipt) is still in flight
            t19 = PAIRS - 1
            mp_l = mpool.tile([128, 2, NP2], DT, tag="mp_last")
            nc.sync.dma_start(
                mp_l[:, 0:1, :NH], mpt_d.ap()[:, 2 * t19:2 * t19 + 1, :NH]
            )
            nc.sync.dma_start(
                mp_l[:, 0:1, NH:], mpt_d.ap()[:, 2 * t19:2 * t19 + 1, NH:]
            )
            nc.gpsimd.memset(mp_l[:, 1:2, :], 0.0)

            # bf16 products: |ff*mf| ~ O(1) and the tanh saturates, so bf16
            # rounding is invisible at the 2e-2 tolerance
            prod = spool.tile([F, NP2], BF16, tag="prod")
            tanh_sb = spool.tile([1, NP2], F32, tag="tanh")
            sums = spool.tile([1, 2], F32, tag="sums")
            ps_dot_a = ppool.tile([1, NH], F32, tag="dotA")
            ps_dot_b = ppool.tile([1, NH], F32, tag="dotB")
            ps_dot = [ps_dot_a, ps_dot_b]
            HALVES = (slice(0, NH), slice(NH, NP2))
            for sl in HALVES:
                nc.tensor.matmul(
                    ps_mf[:, sl],
                    lhsT=wm_sb[:, 2 * t19:2 * t19 + 2, :],
                    rhs=mp_l[:, :, sl],
                    start=False,
                    stop=True,
                    perf_mode=DR,
                    skip_group_check=True,
                )
            for sl in HALVES:
                nc.vector.tensor_tensor(
                    out=prod[:, sl], in0=ff_sb[:, sl], in1=ps_mf[:, sl],
                    op=mybir.AluOpType.mult,
                )
            for h, sl in enumerate(HALVES):
                nc.tensor.matmul(
                    ps_dot[h][:], lhsT=ones_bf[:], rhs=prod[:, sl],
                    start=True, stop=True,
                )
            for h, sl in enumerate(HALVES):
                # tanh + horizontal sum fused in one ACT instruction
                nc.scalar.activation(
                    tanh_sb[:, sl],
                    ps_dot[h][:],
                    mybir.ActivationFunctionType.Tanh,
                    accum_out=sums[:, h:h + 1],
                )
            nc.sync.dma_start(out_d.ap(), sums[:])

    nc.compile()
    return nc


def _get_nc():
    global _COMPILED
    if _COMPILED is None:
        _COMPILED = _build_nc()
    return _COMPILED


def _prep_inputs(fixed, moving, Wf, Wm):
    """Host-side prep: patch-extract to K-major fp8, apply keep mask, pack.

    Returns (per-core input maps, keep_count).
    """

    def vol_to_kmajor(vol):
        # vol [255,255,255] f32 -> [4913, 3375] f32 (K-major patches)
        x = vol.reshape(NPP, PATCH, NPP, PATCH, NPP, PATCH)
        x = x.transpose(1, 3, 5, 0, 2, 4)  # [17,17,17, 15,15,15]
        return np.ascontiguousarray(x).reshape(P3, N_TOT)

    def pad_shard(km8):
        shards = []
        for c in range(N_CORES):
            cols = km8[:, c * NP:min((c + 1) * NP, N_TOT)]
            sh = np.zeros((KPAD, NP2), dtype=NP_DT)
            sh[:P3, :cols.shape[1]] = cols
            # [KPAD, NP2] -> [128, KT, NP2]: partition p holds K rows
            # {t*128+p}, contiguous t-major per partition
            a = sh.reshape(KT, 128, NP2).transpose(1, 0, 2)
            shards.append(np.ascontiguousarray(a))
        return shards

    def pack_w(W):
        wp = np.zeros((KPAD, F), dtype=np.float32)
        wp[:P3] = W
        wp = wp.reshape(KT, 128, F).transpose(1, 0, 2)
        return np.ascontiguousarray(wp.astype(NP_DT))

    fkm = vol_to_kmajor(np.asarray(fixed)[0, 0])    # f32, exact
    mkm = vol_to_kmajor(np.asarray(moving)[0, 0])

    # reference keep mask computed from the exact f32 fixed patches
    zero_cnt = (fkm == 0).sum(axis=0)               # [3375]
    keep = zero_cnt <= ZERO_THRESH * P3
    keep_count = int(keep.sum())

    fkm8 = fkm.astype(NP_DT)
    fkm8[:, ~keep] = 0  # dropped patches contribute exactly 0 to the sum
    mkm8 = mkm.astype(NP_DT)

    fp_shards = pad_shard(fkm8)
    mp_shards = pad_shard(mkm8)
    wf_p = pack_w(np.asarray(Wf))
    wm_p = pack_w(np.asarray(Wm))

    in_maps = [
        {"fpt": fp_shards[c], "mpt": mp_shards[c], "wf": wf_p, "wm": wm_p}
        for c in range(N_CORES)
    ]
    return in_maps, keep_count


def _run(inputs, trace=False, **kwargs):
    nc = _get_nc()
    in_maps, keep_count = _prep_inputs(
        inputs["fixed"], inputs["moving"], inputs["Wf"], inputs["Wm"]
    )
    res = run_bass_kernel_spmd(nc, in_maps, list(range(N_CORES)), trace=trace, **kwargs)
    s = sum(float(np.asarray(r["out"], dtype=np.float64).sum()) for r in res.results)
    value = np.float32(s / max(keep_count, 1.0))
    return np.asarray(value, dtype=np.float32), res


def kernel(**inputs) -> np.ndarray:
    value, _ = _run(inputs, trace=False)
    return value


# revision 28
# speedup vs baseline: 1.0530x; 1.0530x over previous
"""DMMR loss kernel for Trainium2 (8 NeuronCores, data-parallel over patches).

Reference semantics (see problem):
  fp = extract_patches(fixed)   # [3375, 4913]
  mp = extract_patches(moving)  # [3375, 4913]
  keep = (mean(fp == 0, axis=1) <= 0.15)
  out  = tanh(sum((fp @ Wf) * (mp @ Wm), -1))  # [3375]
  value = sum(out * keep) / max(sum(keep), 1)

Sharding: the 3375 patch pairs are split 422-per-core across 8 cores and
padded to 432 columns (16-aligned for the DoubleRow moving AP).  The keep
mask is applied on the host by zeroing the fixed-patch data of dropped
patches (ff=0 -> dot=0 -> tanh=0 contribution, exactly equivalent to
masking); the host also computes the keep count and performs the final
division, so the device computes only sum(tanh(ff . mf)) per shard.

Device layout: K-major fp8 with the contraction dim on SBUF partitions,
DRAM arranged [128, 40 ktiles, 432] so each partition reads contiguous
bytes (3.4KB runs per chunk).  Matmuls use fp8 DoubleRow (K=256 per
instruction): 20 MMs per volume instead of 39.  All data DMAs ride the
Sync HWDGE ring in consumption order (fp chunks then mp chunks); weights
ride the Scalar ring.  Everything stays resident in SBUF, so the rings
drain at full HBM rate and the PE never waits on buffer recycling.
"""

import numpy as np
import ml_dtypes

import concourse.bacc as bacc
import concourse.mybir as mybir
import concourse.tile as tile
from concourse.bass_utils import run_bass_kernel_spmd

PATCH = 17
NPP = 15
N_TOT = NPP**3            # 3375 patches
P3 = PATCH**3             # 4913 elems per patch
F = 64                    # feature dim
N_CORES = 8
NP = 422                  # real patches per core (8*422 = 3376 = 3375 + 1)
NP2 = 432                 # padded to a multiple of 16 (DoubleRow AP step)
KT = 40                   # K tiles of 128 (4913 padded to 5120)
KPAD = KT * 128           # 5120
PAIRS = KT // 2           # 20 DoubleRow pairs (K=256 each)
CT = 8                    # K tiles per DMA chunk (3456B per partition)
NCHUNK = KT // CT         # 5
ZERO_THRESH = 0.15

BF16 = mybir.dt.bfloat16
F32 = mybir.dt.float32
DT = mybir.dt.float8e4
NP_DT = ml_dtypes.float8_e4m3
DR = mybir.MatmulPerfMode.DoubleRow
WARMUP_MM = 4             # throwaway matmuls to pre-warm the PE clock

_COMPILED = None  # cache so repeat kernel() calls reuse the program


def _build_nc():
    nc = bacc.Bacc("TRN2", target_bir_lowering=False, debug=False)

    fpt_d = nc.dram_tensor("fpt", [128, KT, NP2], DT, kind="ExternalInput")
    mpt_d = nc.dram_tensor("mpt", [128, KT, NP2], DT, kind="ExternalInput")
    wf_d = nc.dram_tensor("wf", [128, KT, F], DT, kind="ExternalInput")
    wm_d = nc.dram_tensor("wm", [128, KT, F], DT, kind="ExternalInput")
    out_d = nc.dram_tensor("out", [1, 2], F32, kind="ExternalOutput")

    # All data chunks ride the Sync HWDGE ring in consumption order, so
    # completions arrive in the order the PE consumes them (no FIFO
    # bubbles); weights drain early on the Scalar ring.  K-tile 39 is
    # all padding: its SBUF region is memset instead of transferred.
    # The final pair (19) arrives as two patch-half transfers so half
    # the epilogue chain overlaps the last drain+receipt.
    FP_CHUNKS = [8, 8, 8, 8, 8]
    MP_CHUNKS = [8, 8, 8, 8, 6]     # pairs 0-18; pair 19 is split below
    LAST_TILE = KT - 1  # tile 39: zero-filled, never DMA'd
    WF_SPLIT = 8  # tiles in the first (small) wf piece
    NH = NP2 // 2  # patch-half width (216)

    with tile.TileContext(nc) as tc:
        with (
            tc.tile_pool(name="weights", bufs=1) as wpool,
            tc.tile_pool(name="fdata", bufs=len(FP_CHUNKS)) as fpool,
            tc.tile_pool(name="mdata", bufs=len(MP_CHUNKS)) as mpool,
            tc.tile_pool(name="small", bufs=1) as spool,
            tc.tile_pool(name="psum", bufs=1, space="PSUM") as ppool,
        ):
            # wf arrives in two pieces so the first matmuls only gate on
            # a 64KB transfer; separate tiles give clean dependencies.
            # Everything rides the Sync ring: a single HWDGE ring drains
            # measurably faster than two rings sharing the SDMA engines.
            wfa = wpool.tile([128, WF_SPLIT, F], DT, tag="wfa")
            nc.scalar.dma_start(wfa[:], wf_d.ap()[:, :WF_SPLIT, :])

            def wf_pair(t):
                if 2 * t < WF_SPLIT:
                    return wfa[:, 2 * t:2 * t + 2, :]
                return wfb[:, 2 * t - WF_SPLIT:2 * t - WF_SPLIT + 2, :]

            ones_bf = spool.tile([F, 1], BF16, tag="ones_bf")
            nc.vector.memset(ones_bf[:], 1.0)
            junk = spool.tile([128, 2, NP2], DT, tag="junk")
            nc.vector.memset(junk[:], 0.0)

            ps_ff = ppool.tile([F, NP2], F32, tag="ff")
            ps_mf = ppool.tile([F, NP2], F32, tag="mf")
            ps_warm = ppool.tile([F, NP2], F32, tag="warm")

            # pre-warm the PE HAM clock gate during the initial DMA wait
            for w in range(WARMUP_MM):
                nc.tensor.matmul(
                    ps_warm[:],
                    lhsT=junk[:, :, :F],
                    rhs=junk[:],
                    start=(w == 0),
                    stop=(w == WARMUP_MM - 1),
                    perf_mode=DR,
                )

            def stream_volume(chunks, dram, pool, tag, psum, lhsT_of, last_stop,
                              inject=None):
                t = 0
                off = 0
                for ci, ct in enumerate(chunks):
                    ch = pool.tile([128, ct, NP2], DT, tag=tag)
                    if off + ct > LAST_TILE:
                        # last chunk: transfer all but tile 39, memset it
                        nc.sync.dma_start(
                            ch[:, :ct - 1, :],
                            dram.ap()[:, off:off + ct - 1, :],
                        )
                        nc.gpsimd.memset(ch[:, ct - 1:ct, :], 0.0)
                    else:
                        nc.sync.dma_start(
                            ch[:], dram.ap()[:, off:off + ct, :]
                        )
                    if inject and ci in inject:
                        inject[ci]()
                    for s in range(ct // 2):
                        nc.tensor.matmul(
                            psum[:],
                            lhsT=lhsT_of(t),
                            rhs=ch[:, 2 * s:2 * s + 2, :],
                            start=(t == 0),
                            stop=(t == last_stop),
                            perf_mode=DR,
                        )
                        t += 1
                    off += ct

            wfb = wpool.tile([128, KT - WF_SPLIT, F], DT, tag="wfb")
            nc.scalar.dma_start(wfb[:], wf_d.ap()[:, WF_SPLIT:, :])
            wm_sb = wpool.tile([128, KT, F], DT, tag="wm")
            nc.scalar.dma_start(wm_sb[:], wm_d.ap())

            # ---- phase 1: fixed volume (ff DoubleRow matmuls) ----
            stream_volume(
                FP_CHUNKS, fpt_d, fpool, "fp", ps_ff, wf_pair, PAIRS - 1,
            )

            # stage ff out of PSUM on the (idle) DVE so the Scalar queue
            # never head-of-line-blocks the scalar ring's descriptor gen
            ff_sb = spool.tile([F, NP2], F32, tag="ff_sb")
            nc.vector.tensor_scalar(
                out=ff_sb[:], in0=ps_ff[:], scalar1=0.0, scalar2=None,
                op0=mybir.AluOpType.add,
            )

            # ---- phase 2: moving volume, pairs 0-18 ----
            stream_volume(MP_CHUNKS, mpt_d, mpool, "mp", ps_mf,
                          lambda t: wm_sb[:, 2 * t:2 * t + 2, :], -1)

            # pair 19 arrives as two patch-half DMAs; each half's epilogue
            # chain (matmul -> prod -> dot -> tanh) runs while the other
            # half (and the output receipt) is still in flight
            t19 = PAIRS - 1
            mp_l = mpool.tile([128, 2, NP2], DT, tag="mp_last")
            nc.sync.dma_start(
                mp_l[:, 0:1, :NH], mpt_d.ap()[:, 2 * t19:2 * t19 + 1, :NH]
            )
            nc.sync.dma_start(
                mp_l[:, 0:1, NH:], mpt_d.ap()[:, 2 * t19:2 * t19 + 1, NH:]
            )
            nc.gpsimd.memset(mp_l[:, 1:2, :], 0.0)

            # bf16 products: |ff*mf| ~ O(1) and the tanh saturates, so bf16
            # rounding is invisible at the 2e-2 tolerance
            prod = spool.tile([F, NP2], BF16, tag="prod")
            tanh_sb = spool.tile([1, NP2], F32, tag="tanh")
            sums = spool.tile([1, 2], F32, tag="sums")
            ps_dot_a = ppool.tile([1, NH], F32, tag="dotA")
            ps_dot_b = ppool.tile([1, NH], F32, tag="dotB")
            ps_dot = [ps_dot_a, ps_dot_b]
            HALVES = (slice(0, NH), slice(NH, NP2))
            for sl in HALVES:
                nc.tensor.matmul(
                    ps_mf[:, sl],
                    lhsT=wm_sb[:, 2 * t19:2 * t19 + 2, :],
                    rhs=mp_l[:, :, sl],
                    start=False,
                    stop=True,
                    perf_mode=DR,
                    skip_group_check=True,
                )
            for sl in HALVES:
                nc.vector.tensor_tensor(
                    out=prod[:, sl], in0=ff_sb[:, sl], in1=ps_mf[:, sl],
                    op=mybir.AluOpType.mult,
                )
            for h, sl in enumerate(HALVES):
                nc.tensor.matmul(
                    ps_dot[h][:], lhsT=ones_bf[:], rhs=prod[:, sl],
                    start=True, stop=True,
                )
            for h, sl in enumerate(HALVES):
                # tanh + horizontal sum fused in one ACT instruction
                nc.scalar.activation(
                    tanh_sb[:, sl],
                    ps_dot[h][:],
                    mybir.ActivationFunctionType.Tanh,
                    accum_out=sums[:, h:h + 1],
                )
            nc.scalar.dma_start(out_d.ap(), sums[:])

    nc.compile()
    return nc


def _get_nc():
    global _COMPILED
    if _COMPILED is None:
        _COMPILED = _build_nc()
    return _COMPILED


def _prep_inputs(fixed, moving, Wf, Wm):
    """Host-side prep: patch-extract to K-major fp8, apply keep mask, pack.

    Returns (per-core input maps, keep_count).
    """

    def vol_to_kmajor(vol):
        # vol [255,255,255] f32 -> [4913, 3375] f32 (K-major patches)
        x = vol.reshape(NPP, PATCH, NPP, PATCH, NPP, PATCH)
        x = x.transpose(1, 3, 5, 0, 2, 4)  # [17,17,17, 15,15,15]
        return np.ascontiguousarray(x).reshape(P3, N_TOT)

    def pad_shard(km8):
        shards = []
        for c in range(N_CORES):
            cols = km8[:, c * NP:min((c + 1) * NP, N_TOT)]
            sh = np.zeros((KPAD, NP2), dtype=NP_DT)
            sh[:P3, :cols.shape[1]] = cols
            # [KPAD, NP2] -> [128, KT, NP2]: partition p holds K rows
            # {t*128+p}, contiguous t-major per partition
            a = sh.reshape(KT, 128, NP2).transpose(1, 0, 2)
            shards.append(np.ascontiguousarray(a))
        return shards

    def pack_w(W):
        wp = np.zeros((KPAD, F), dtype=np.float32)
        wp[:P3] = W
        wp = wp.reshape(KT, 128, F).transpose(1, 0, 2)
        return np.ascontiguousarray(wp.astype(NP_DT))

    fkm = vol_to_kmajor(np.asarray(fixed)[0, 0])    # f32, exact
    mkm = vol_to_kmajor(np.asarray(moving)[0, 0])

    # reference keep mask computed from the exact f32 fixed patches
    zero_cnt = (fkm == 0).sum(axis=0)               # [3375]
    keep = zero_cnt <= ZERO_THRESH * P3
    keep_count = int(keep.sum())

    fkm8 = fkm.astype(NP_DT)
    fkm8[:, ~keep] = 0  # dropped patches contribute exactly 0 to the sum
    mkm8 = mkm.astype(NP_DT)

    fp_shards = pad_shard(fkm8)
    mp_shards = pad_shard(mkm8)
    wf_p = pack_w(np.asarray(Wf))
    wm_p = pack_w(np.asarray(Wm))

    in_maps = [
        {"fpt": fp_shards[c], "mpt": mp_shards[c], "wf": wf_p, "wm": wm_p}
        for c in range(N_CORES)
    ]
    return in_maps, keep_count


def _run(inputs, trace=False, **kwargs):
    nc = _get_nc()
    in_maps, keep_count = _prep_inputs(
        inputs["fixed"], inputs["moving"], inputs["Wf"], inputs["Wm"]
    )
    res = run_bass_kernel_spmd(nc, in_maps, list(range(N_CORES)), trace=trace, **kwargs)
    s = sum(float(np.asarray(r["out"], dtype=np.float64).sum()) for r in res.results)
    value = np.float32(s / max(keep_count, 1.0))
    return np.asarray(value, dtype=np.float32), res


def kernel(**inputs) -> np.ndarray:
    value, _ = _run(inputs, trace=False)
    return value
